# revision 33
# baseline (speedup 1.0000x reference)
"""GQA attention kernel for Trainium2, data-parallel over batch on 8 NeuronCores.

Per-core problem (2 of 16 batches): X [1024tok, 1024] -> QKV proj -> RoPE ->
causal GQA attention (8 q heads, 4 kv heads, D=128) -> out proj [1024, 1024].

v3 layout strategy (PE-occupancy-driven; baseline v2 was ~146.4us):
  - All dram tensors are host-side pre-arranged to the exact sbuf fill
    layout ([p, k, n]) so every load DMA runs with 4-16KB contiguous
    descriptors, and loads are ordered by first use with the K-projection
    dependencies (X, then per-g Wk chunks) first across both HWDGE rings.
  - RoPE's partition-half swap is done ON THE DVE via stream_shuffle: the
    head dim of Wq/Wk (and cos/sin rows) is permuted so each rotate-half
    pair (i, i+64) lands 16 partitions apart inside one 32-partition
    quadrant (S = q.k is invariant under a consistent d-permutation).
    This removes all sbuf<->sbuf swap DMAs (3MB of ring traffic that used
    to compete with the weight loads) and makes rope a pure DVE chain.
  - The 1/sqrt(D) scale is folded into the exp's activation scale, so one
    UNSCALED cos/sin table pair [128, 512] is shared by Q and K rope and
    broadcast over the two batch chunks (0.25MB loaded vs 1MB).
  - Softmax denominator: the colsum matmul uses an ALL-ONES [128,128]
    stationary operand, so the psum result is the denominator already
    broadcast across partitions (same PE streaming cost, cheaper
    instruction shape than M=1) and normalization is a DVE
    reciprocal_approx_fast + one multiply -- no GPSIMD
    partition_broadcast; GPSIMD retires from the kernel entirely.
    (A single tensor_tensor divide would be cheaper still but the BIR
    verifier rejects divide on the DVE.)
  - PE warmup is dependency-light (ones@ones after a gpsimd memset) so it
    starts as soon as the PE queue comes up (~6.6us) and is sized to end
    exactly when X+Wk land (~15us), covering the whole load latency.
  - Output dram tensor is bf16 (host upcasts): halves store traffic; all
    stores issue from the idle sync queue (a dma_start occupies its
    issuing engine ~1us), and the final chunk drains in [128,128]
    quarters, copies (DVE/ACT alternating) emitted before any store.
  - Engine assignment is FIFO-queue-driven (strict in-order queues):
      ACT:    exps, psum->sbuf copies (qraw/V/out tiles)
      DVE:    rope (shuffle+mul+mul+add), merged causal masks, divides
      GPSIMD: nothing (only the startup ones memset)
      sync/scalar DMA rings: loads first-use-ordered, then output stores
  - Same software pipeline as v2: S/exp emission runs a full head ahead of
    the colsum/PV consumers (deque, drain threshold 11), attention
    consumption lags Q-chain emission by 3 heads, rope DVE chains are
    emitted one head-slot late.
"""

import numpy as np
import ml_dtypes
from collections import deque
from contextlib import ExitStack

import concourse.bass as bass
import concourse.tile as tile
from concourse import bacc, mybir
from concourse.bass_utils import run_bass_kernel_spmd

B, T, HID = 16, 512, 1024
NH, NKV, D = 8, 4, 128
THETA = 10000.0
NCORES = 8
BL = B // NCORES          # local batches per core
TOK = BL * T              # local tokens
P = 128
KT_HID = HID // P         # 8 contraction tiles over hidden
NTQ = T // P              # 4 tk/tq tiles per sequence
NTOK_T = TOK // P         # 8 token tiles per core
GROUPS = NH // NKV        # 2 q heads per kv head
SCALE = 1.0 / float(np.sqrt(D))
FP32 = mybir.dt.float32
BF16 = mybir.dt.bfloat16
BF = ml_dtypes.bfloat16

# rope-pair permutation: old pair (i, i+64) -> within-quadrant pair
# (32q+j, 32q+16+j) with q=i//16, j=i%16, so one stream_shuffle mask
# (swap 16-partition halves of each 32-partition quadrant) does the
# rotate-half partition move on the DVE.
_DPERM = np.empty(D, dtype=np.int64)          # old index of each new slot
for _q in range(4):
    for _j in range(16):
        _DPERM[32 * _q + _j] = 16 * _q + _j
        _DPERM[32 * _q + 16 + _j] = 64 + 16 * _q + _j
SHUF_MASK = list(range(16, 32)) + list(range(0, 16))

WARM = 56                 # PE warmup matmuls (ones[P,P] @ ones[P,P])


def _host_consts():
    inv_freq = 1.0 / (THETA ** (np.arange(0, D, 2, dtype=np.float64) / D))
    freqs = np.outer(np.arange(T, dtype=np.float64), inv_freq)    # [T, 64]
    emb = np.concatenate([freqs, freqs], axis=-1)                 # [T, 128]
    cos = np.cos(emb).T                                           # [128, T]
    sin = np.sin(emb).T
    # rotate_half sign folded into sin: out = x*cos + shuffle(x)*sin_signed
    sin_signed = np.concatenate([-sin[:D // 2], sin[D // 2:]], axis=0)
    # transposed-S diagonal-block multiplicative mask: rows tk, cols tq;
    # valid iff tq >= tk
    mask_t = np.triu(np.ones((P, P), np.float32)).astype(BF)
    return {
        "cos_t": np.ascontiguousarray(cos[_DPERM]).astype(BF),
        "sin_t": np.ascontiguousarray(sin_signed[_DPERM]).astype(BF),
        "mask_t": mask_t,
    }


def _build(nc):
    hid = nc.dram_tensor("hidden_pk", [P, KT_HID, TOK], BF16,
                         kind="ExternalInput").ap()
    wq = nc.dram_tensor("wq_pk", [P, KT_HID, NH * D], BF16,
                        kind="ExternalInput").ap()
    wk = nc.dram_tensor("wk_g", [NKV, P, KT_HID, D], BF16,
                        kind="ExternalInput").ap()
    wv = nc.dram_tensor("wv_pk", [P, KT_HID, NKV * D], BF16,
                        kind="ExternalInput").ap()
    wo = nc.dram_tensor("wo_pk", [P, KT_HID, HID], BF16,
                        kind="ExternalInput").ap()
    cos_t = nc.dram_tensor("cos_t", [P, T], BF16, kind="ExternalInput").ap()
    sin_t = nc.dram_tensor("sin_t", [P, T], BF16, kind="ExternalInput").ap()
    mask_t = nc.dram_tensor("mask_t", [P, P], BF16, kind="ExternalInput").ap()
    out = nc.dram_tensor("out", [TOK, HID], BF16, kind="ExternalOutput").ap()

    with tile.TileContext(nc) as tc, ExitStack() as ctx:
        # ---- pools with cross-phase lifetimes ----
        consts = ctx.enter_context(tc.tile_pool(name="consts", bufs=1))

        cos_sb = consts.tile([P, T], BF16, tag="cos")
        sin_sb = consts.tile([P, T], BF16, tag="sin")
        maskt_sb = consts.tile([P, P], BF16, tag="maskt")
        ones_bf = consts.tile([P, P], BF16, tag="ones")
        # gpsimd comes up first (~6.1us) -> warmup deps ready earliest
        nc.gpsimd.memset(ones_bf, 1.0)

        qkvpool = ctx.enter_context(tc.tile_pool(name="qkv", bufs=1))
        qt_sb = qkvpool.tile([P, NH, BL, T], BF16, tag="qt")    # [d,h,b,t]
        kt_sb = qkvpool.tile([P, NKV, BL, T], BF16, tag="kt")   # [d,g,b,t]
        v_sb = qkvpool.tile([P, NTOK_T, NKV * D], BF16, tag="v")
        otpool = ctx.enter_context(tc.tile_pool(name="otpool", bufs=1))
        ot_sb = otpool.tile([P, NH, BL, T], BF16, tag="ot")     # [d,h,b,t]

        # single PSUM pool for BOTH phases: phase A draws its K-wave/V-proj
        # psums from the same tag rotations phase B uses, so there is no
        # psum pool close/open barrier between the phases
        psAll = ctx.enter_context(
            tc.tile_pool(name="psAll", bufs=1, space=bass.MemorySpace.PSUM))

        wpool = ctx.enter_context(tc.tile_pool(name="wpool", bufs=1))
        wq_sb = wpool.tile([P, KT_HID, NH * D], BF16, tag="wq")
        wk_sb = wpool.tile([P, NKV, KT_HID, D], BF16, tag="wk")
        wv_sb = wpool.tile([P, KT_HID, NKV * D], BF16, tag="wv")
        wo_sb = wpool.tile([P, KT_HID, HID], BF16, tag="wo")
        xt_sb = wpool.tile([P, KT_HID, TOK], BF16, tag="xt")    # [hid,k,tok]

        # ---- input loads: first-use order, aware that the DMA engines
        # alternate between the two rings' FIFOs (global completion order
        # ~= zip of the two lists) and that a completed transfer is only
        # CONSUMABLE ~2.5us later (completion->semaphore->engine latency).
        # Wave 1 of the K proj needs wk0+wk1+xt0 first; wk2/wk3 are only
        # needed when wave 2 starts (~7us later) ----
        nc.sync.dma_start(out=wk_sb[:, 0], in_=wk[0])
        nc.scalar.dma_start(out=wk_sb[:, 1], in_=wk[1])
        for k in range(KT_HID):
            eng = nc.sync if k % 2 == 0 else nc.scalar
            eng.dma_start(out=xt_sb[:, k, :], in_=hid[:, k, :])
        nc.sync.dma_start(out=wk_sb[:, 2], in_=wk[2])
        nc.scalar.dma_start(out=wk_sb[:, 3], in_=wk[3])
        nc.sync.dma_start(out=cos_sb, in_=cos_t)
        nc.scalar.dma_start(out=sin_sb, in_=sin_t)
        nc.scalar.dma_start(out=wv_sb, in_=wv)
        nc.sync.dma_start(out=wq_sb[:, 0:4, :], in_=wq[:, 0:4, :])
        nc.scalar.dma_start(out=wq_sb[:, 4:8, :], in_=wq[:, 4:8, :])
        nc.sync.dma_start(out=maskt_sb, in_=mask_t)
        nc.sync.dma_start(out=wo_sb[:, 0:4, :], in_=wo[:, 0:4, :])
        nc.scalar.dma_start(out=wo_sb[:, 4:8, :], in_=wo[:, 4:8, :])

        cos_bc = cos_sb[:, None, :].to_broadcast([P, BL, T])
        sin_bc = sin_sb[:, None, :].to_broadcast([P, BL, T])

        def _rope_dve(raw, out_sl, tmp_pool):
            """Full rope on the DVE: partition-half swap via stream_shuffle
            (head-dim permuted so pairs sit within 32-partition quadrants),
            then out = raw*cos + shuffled*sin_signed."""
            swp = tmp_pool.tile([P, BL, T], BF16, tag="rope_swp", bufs=2,
                                name="swp")
            nc.vector.stream_shuffle(swp, raw, SHUF_MASK)
            tmp = tmp_pool.tile([P, BL, T], BF16, tag="rope_tmp", bufs=2,
                                name="tmp")
            nc.vector.tensor_mul(tmp, swp, sin_bc)
            nc.vector.tensor_mul(out_sl, raw, cos_bc)
            nc.vector.tensor_add(out_sl, out_sl, tmp)

        # explicit phase-B psum tags for phase A's draws, chosen so no
        # draw ever WARs against a still-draining prior user: warmup=qps#1,
        # wave1=[qps#2,sps#1-3], wave2=[ops#1-2,cps#1,qps#1(warmup's,
        # long free)], V proj cycles slots whose copies drained chains ago
        _atags = iter([("qps", 2), ("qps", 2), ("sps", 3), ("sps", 3),
                       ("sps", 3), ("ops", 2), ("ops", 2), ("cps", 1),
                       ("qps", 2), ("sps", 3), ("sps", 3), ("sps", 3),
                       ("ops", 2), ("ops", 2), ("cps", 1), ("sps", 3),
                       ("ops", 2)])

        def _psA_tile():
            tag, bufs = next(_atags)
            return psAll.tile([P, T], FP32, tag=tag, bufs=bufs, name="aps")

        # ---- phase A: warmup + K proj + V proj ----
        with ExitStack() as phase1:
            ropet = phase1.enter_context(tc.tile_pool(name="ropetA", bufs=2))

            # PE warmup: dependency-free matmuls so the PE queue ramps the
            # HAM clock gate (1.2 -> 2.4 GHz) while Wk + the first X planes
            # are in flight; sized to end right as they land (~11.5us)
            wps = _psA_tile()
            for w in range(WARM):
                nc.tensor.matmul(wps[:, 0:P], ones_bf, ones_bf,
                                 start=True, stop=True, skip_group_check=True)

            # KT = Wk.T @ XT: the 8 (g, c) chains run as two 4-chain waves
            # interleaved plane-by-plane so the PE chases the X DMA
            # wavefront (plane k feeds 4 matmuls ~0.85us while the next
            # plane needs ~0.7us to load); wave 1 only needs wk0/wk1 so it
            # can start as soon as xt0 is consumable
            kraws = [ropet.tile([P, BL, T], BF16, tag="rope_raw", bufs=4,
                                name="kraw") for _ in range(NKV)]
            for wave in range(2):
                gs = (0, 1) if wave == 0 else (2, 3)
                kps = {(g, c): _psA_tile()
                       for g in gs for c in range(BL)}
                for k in range(KT_HID):
                    for (g, c), ps in kps.items():
                        nc.tensor.matmul(
                            ps,
                            wk_sb[:, g, k, :],
                            xt_sb[:, k, c * T:(c + 1) * T],
                            start=(k == 0), stop=(k == KT_HID - 1),
                            skip_group_check=True)
                for g in gs:
                    for c in range(BL):
                        nc.scalar.copy(kraws[g][:, c, :], kps[(g, c)])
                    _rope_dve(kraws[g], kt_sb[:, g], ropet)
            # V natural: [tok, dkv]
            for tt in range(NTOK_T):
                ps = _psA_tile()
                for k in range(KT_HID):
                    nc.tensor.matmul(
                        ps[:, :NKV * D],
                        xt_sb[:, k, tt * P:(tt + 1) * P],
                        wv_sb[:, k, :],
                        start=(k == 0), stop=(k == KT_HID - 1))
                nc.scalar.copy(v_sb[:, tt, :], ps[:, :NKV * D])

        # ---- phase B: Q proj interleaved with attention ----
        with ExitStack() as phase2:
            ropet = phase2.enter_context(tc.tile_pool(name="ropetB", bufs=2))
            ptpool = phase2.enter_context(tc.tile_pool(name="ptpool", bufs=5))
            stats = phase2.enter_context(tc.tile_pool(name="stats", bufs=3))
            psM = psAll

            pend = deque()
            head_state = {}
            qraw_state = {}
            rope_pending = deque()

            def emit_qchain(h, c):
                ps = psM.tile([P, T], FP32, tag="qps", bufs=2, name="qps")
                for k in range(KT_HID):
                    nc.tensor.matmul(
                        ps,
                        wq_sb[:, k, h * P:(h + 1) * P],
                        xt_sb[:, k, c * T:(c + 1) * T],
                        start=(k == 0), stop=(k == KT_HID - 1))
                if c == 0:
                    qraw_state[h] = ropet.tile([P, BL, T], BF16,
                                               tag="rope_raw", bufs=2,
                                               name="qraw")
                qraw = qraw_state[h]
                nc.scalar.copy(qraw[:, c, :], ps)
                if c == BL - 1:
                    rope_pending.append((qraw, h))
                    del qraw_state[h]

            def emit_rope_dve():
                qraw, h = rope_pending.popleft()
                _rope_dve(qraw, qt_sb[:, h], ropet)

            def emit_item_S(b, h):
                """S matmuls + exps for all 4 tk-blocks of one (batch, head),
                then ONE merged causal-mask multiply: each PT_j plane stores
                columns [lo:T] shifted to plane-local [0:T-lo], so all four
                diagonal blocks line up at plane-local [0:P] and mask in a
                single strided DVE op."""
                g = h // GROUPS
                pt = ptpool.tile([P, NTQ, T], BF16, tag="pt", bufs=5,
                                 name="pt")
                for j in range(NTQ):
                    lo = j * P
                    st_ps = psM.tile([P, T], FP32, tag="sps", bufs=3,
                                     name="sps")
                    nc.tensor.matmul(
                        st_ps[:, lo:T],
                        kt_sb[:, g, b, lo:lo + P],
                        qt_sb[:, h, b, lo:T],
                        start=True, stop=True)
                    # exp -> PT_j, already transposed for the PV matmul;
                    # 1/sqrt(D) folded into the activation scale
                    # (no row-max: logits are O(1) by construction)
                    nc.scalar.activation(
                        out=pt[:, j, 0:T - lo], in_=st_ps[:, lo:T],
                        func=mybir.ActivationFunctionType.Exp,
                        bias=0.0, scale=SCALE)
                nc.vector.tensor_mul(
                    pt[:, :, 0:P], pt[:, :, 0:P],
                    maskt_sb[:, None, :].to_broadcast([P, NTQ, P]))
                for j in range(NTQ):
                    pend.append((b, h, j, pt))

            def drain_one():
                b, h, j, pt = pend.popleft()
                g = h // GROUPS
                lo = j * P
                st = head_state.get((b, h))
                if st is None:
                    o_ps_new = psM.tile([P, T], FP32, tag="ops", bufs=2,
                                        name="ops")
                    cs_ps_new = psM.tile([P, T], FP32, tag="cps", bufs=1,
                                         name="cps")
                    st = head_state[(b, h)] = (o_ps_new, cs_ps_new)
                o_ps, cs_ps = st
                # colsum with an ALL-ONES stationary: psum rows all carry
                # the denominator (partition-broadcast for free);
                # OT += V_j.T @ PT_j chained in the other psum
                nc.tensor.matmul(
                    cs_ps[:, lo:T] if j else cs_ps[:, :],
                    ones_bf,
                    pt[:, j, 0:T - lo],
                    start=(j == 0), stop=(j == NTQ - 1),
                    skip_group_check=True)
                nc.tensor.matmul(
                    o_ps[:, lo:T] if j else o_ps[:, :],
                    v_sb[:, b * NTQ + j, g * D:(g + 1) * D],
                    pt[:, j, 0:T - lo],
                    start=(j == 0), stop=(j == NTQ - 1),
                    skip_group_check=True)
                if j == NTQ - 1:
                    # per-head softmax normalization, fully on the DVE: the
                    # all-ones colsum already broadcast the denominator
                    # across partitions, so reciprocal + one multiply
                    rb = stats.tile([P, T], FP32, tag="rb", bufs=2,
                                    name="rb")
                    nc.vector.reciprocal_approx_fast(rb, cs_ps)
                    nc.vector.tensor_mul(ot_sb[:, h, b], o_ps, rb)
                    del head_state[(b, h)]

            def emit_att(b, h):
                emit_item_S(b, h)
                while len(pend) > 11:
                    drain_one()

            def emit_oproj(tt, cchunk):
                # one out-projection chunk: out[tt-block, chunk] as its own
                # k-chain, sharing the qps psum rotation with the (finished)
                # Q chains so it can interleave with the attention tail
                ps = psM.tile([P, T], FP32, tag="qps", bufs=2, name="opps")
                for k in range(KT_HID):
                    nc.tensor.matmul(
                        ps,
                        ot_sb[:, k, tt // NTQ,
                              (tt % NTQ) * P:(tt % NTQ + 1) * P],
                        wo_sb[:, k, cchunk * T:(cchunk + 1) * T],
                        start=(k == 0), stop=(k == KT_HID - 1))
                o_tile = stats.tile([P, T], BF16, tag="oout", bufs=3,
                                    name="o_tile")
                if tt == NTOK_T - 1 and cchunk == 1:
                    # very last chunk: drain as TWO halves, one per copy
                    # engine, running concurrently -- psum-read copies are
                    # latency-bound (~0.6us effective each regardless of
                    # width), so fewer, wider copies beat quarters; copies
                    # emitted before stores so no dma_start delays a copy
                    HT = T // 2
                    base = cchunk * T
                    nc.vector.tensor_copy(o_tile[:, 0:HT], ps[:, 0:HT])
                    nc.scalar.copy(o_tile[:, HT:T], ps[:, HT:T])
                    nc.sync.dma_start(
                        out=out[tt * P:(tt + 1) * P, base:base + HT],
                        in_=o_tile[:, 0:HT])
                    nc.scalar.dma_start(
                        out=out[tt * P:(tt + 1) * P, base + HT:base + T],
                        in_=o_tile[:, HT:T])
                    return
                # copies always on ACT: a PE-dependent copy on the DVE would
                # head-of-line delay the recips/norms that gate later chains;
                # stores always on the otherwise-idle sync queue
                nc.scalar.copy(o_tile, ps)
                nc.sync.dma_start(
                    out=out[tt * P:(tt + 1) * P,
                            cchunk * T:(cchunk + 1) * T],
                    in_=o_tile)

            # attention lags the Q chains by 3 heads and the DVE part of
            # each rope lags its chain by 1 slot: every engine-queue entry
            # has its dependencies resolved before it reaches the queue
            # head, so the strict-FIFO DVE queue never head-of-line blocks
            # the causal masks that gate the PE's colsum/PV matmuls
            LAG = 3
            for h in range(NH):
                if h >= LAG:
                    emit_att(0, h - LAG)
                emit_qchain(h, 0)
                if h >= LAG:
                    emit_att(1, h - LAG)
                if rope_pending and h >= 1:
                    emit_rope_dve()
                emit_qchain(h, 1)
            while rope_pending:
                emit_rope_dve()
            # tail: batch-0 attention first, then interleave out-projection
            # chunks (batch-0 token tiles first) with the remaining
            # ACT/DVE-bound attention so the PE stays dense
            for h in range(NH - LAG, NH):
                emit_att(0, h)
            emit_att(1, NH - LAG)
            while len(pend) > 4:
                drain_one()              # flush: norms (0, *) all emitted
            emit_oproj(0, 0)
            emit_oproj(0, 1)
            emit_att(1, NH - 2)
            emit_oproj(1, 0)
            emit_oproj(1, 1)
            emit_att(1, NH - 1)
            emit_oproj(2, 0)
            emit_oproj(2, 1)
            while len(pend) > 4:
                drain_one()              # flush: norm (1, NH-2) emitted
            emit_oproj(3, 0)
            while pend:
                drain_one()              # norm (1, NH-1)
            emit_oproj(3, 1)
            for tt in range(4, NTOK_T):
                emit_oproj(tt, 0)
                emit_oproj(tt, 1)


def _prep_weights(Wq, Wk, Wv, Wo):
    """Host-side: bf16, rope-pair permutation of the q/k head dims, and
    rearrangement to the sbuf fill layout [p, k, n] (contiguous per
    partition) so load DMA descriptors are large."""
    wq = np.asarray(Wq, dtype=np.float32).astype(BF)
    wk = np.asarray(Wk, dtype=np.float32).astype(BF)
    wv = np.asarray(Wv, dtype=np.float32).astype(BF)
    wo = np.asarray(Wo, dtype=np.float32).astype(BF)
    # permute the per-head d axis of Wq/Wk
    wq = wq.reshape(HID, NH, D)[:, :, _DPERM].reshape(HID, NH * D)
    wk = wk.reshape(HID, NKV, D)[:, :, _DPERM].reshape(HID, NKV * D)
    wq_pk = np.ascontiguousarray(
        wq.reshape(KT_HID, P, NH * D).transpose(1, 0, 2))
    wk_g = np.ascontiguousarray(
        wk.reshape(KT_HID, P, NKV, D).transpose(2, 1, 0, 3))
    wv_pk = np.ascontiguousarray(
        wv.reshape(KT_HID, P, NKV * D).transpose(1, 0, 2))
    wo_pk = np.ascontiguousarray(
        wo.reshape(KT_HID, P, HID).transpose(1, 0, 2))
    return {"wq_pk": wq_pk, "wk_g": wk_g, "wv_pk": wv_pk, "wo_pk": wo_pk}


_COMPILED = None


def _get_compiled():
    global _COMPILED
    if _COMPILED is None:
        nc = bacc.Bacc("TRN2", target_bir_lowering=False, debug=False)
        _build(nc)
        nc.compile()
        _COMPILED = nc
    return _COMPILED


def kernel(hidden_states, Wq, Wk, Wv, Wo, _trace=False, _trace_kwargs=None):
    hs = np.asarray(hidden_states, dtype=np.float32).astype(BF)
    weights = _prep_weights(Wq, Wk, Wv, Wo)
    consts = _host_consts()
    nc = _get_compiled()
    in_maps = []
    for c in range(NCORES):
        # ship X pre-transposed AND pre-tiled ([p, k, tok]) so the kernel's
        # loads are plane-sliced contiguous DMAs
        shard = hs[BL * c: BL * (c + 1)].reshape(TOK, HID).T  # [HID, TOK]
        shard_pk = np.ascontiguousarray(
            shard.reshape(KT_HID, P, TOK).transpose(1, 0, 2))
        in_maps.append({"hidden_pk": shard_pk, **weights, **consts})
    res = run_bass_kernel_spmd(
        nc, in_maps, list(range(NCORES)), trace=_trace,
        **(_trace_kwargs or {}))
    outs = [np.asarray(r["out"]).astype(np.float32).reshape(BL, T, HID)
            for r in res.results]
    full = np.concatenate(outs, axis=0)
    if _trace:
        return full, res
    return full


# revision 36
# speedup vs baseline: 1.0331x; 1.0331x over previous
"""GQA attention kernel for Trainium2, data-parallel over batch on 8 NeuronCores.

Per-core problem (2 of 16 batches): X [1024tok, 1024] -> QKV proj -> RoPE ->
causal GQA attention (8 q heads, 4 kv heads, D=128) -> out proj [1024, 1024].

v3 layout strategy (PE-occupancy-driven; baseline v2 was ~146.4us):
  - All dram tensors are host-side pre-arranged to the exact sbuf fill
    layout ([p, k, n]) so every load DMA runs with 4-16KB contiguous
    descriptors, and loads are ordered by first use with the K-projection
    dependencies (X, then per-g Wk chunks) first across both HWDGE rings.
  - RoPE's partition-half swap is done ON THE DVE via stream_shuffle: the
    head dim of Wq/Wk (and cos/sin rows) is permuted so each rotate-half
    pair (i, i+64) lands 16 partitions apart inside one 32-partition
    quadrant (S = q.k is invariant under a consistent d-permutation).
    This removes all sbuf<->sbuf swap DMAs (3MB of ring traffic that used
    to compete with the weight loads) and makes rope a pure DVE chain.
  - The 1/sqrt(D) scale is folded into the exp's activation scale, so one
    UNSCALED cos/sin table pair [128, 512] is shared by Q and K rope and
    broadcast over the two batch chunks (0.25MB loaded vs 1MB).
  - Softmax denominator: the colsum matmul uses an ALL-ONES [128,128]
    stationary operand, so the psum result is the denominator already
    broadcast across partitions (same PE streaming cost, cheaper
    instruction shape than M=1) and normalization is a DVE
    reciprocal_approx_fast + one multiply -- no GPSIMD
    partition_broadcast; GPSIMD retires from the kernel entirely.
    (A single tensor_tensor divide would be cheaper still but the BIR
    verifier rejects divide on the DVE.)
  - PE warmup is dependency-light (ones@ones after a gpsimd memset) so it
    starts as soon as the PE queue comes up (~6.6us) and is sized to end
    exactly when X+Wk land (~15us), covering the whole load latency.
  - Output dram tensor is bf16 (host upcasts): halves store traffic; all
    stores issue from the idle sync queue (a dma_start occupies its
    issuing engine ~1us), and the final chunk drains in [128,128]
    quarters, copies (DVE/ACT alternating) emitted before any store.
  - Engine assignment is FIFO-queue-driven (strict in-order queues):
      ACT:    exps, psum->sbuf copies (qraw/V/out tiles)
      DVE:    rope (shuffle+mul+mul+add), merged causal masks, divides
      GPSIMD: nothing (only the startup ones memset)
      sync/scalar DMA rings: loads first-use-ordered, then output stores
  - Same software pipeline as v2: S/exp emission runs a full head ahead of
    the colsum/PV consumers (deque, drain threshold 11), attention
    consumption lags Q-chain emission by 3 heads, rope DVE chains are
    emitted one head-slot late.
"""

import numpy as np
import ml_dtypes
from collections import deque
from contextlib import ExitStack

import concourse.bass as bass
import concourse.tile as tile
from concourse import bacc, mybir
from concourse.bass_utils import run_bass_kernel_spmd

B, T, HID = 16, 512, 1024
NH, NKV, D = 8, 4, 128
THETA = 10000.0
NCORES = 8
BL = B // NCORES          # local batches per core
TOK = BL * T              # local tokens
P = 128
KT_HID = HID // P         # 8 contraction tiles over hidden
NTQ = T // P              # 4 tk/tq tiles per sequence
NTOK_T = TOK // P         # 8 token tiles per core
GROUPS = NH // NKV        # 2 q heads per kv head
SCALE = 1.0 / float(np.sqrt(D))
FP32 = mybir.dt.float32
BF16 = mybir.dt.bfloat16
BF = ml_dtypes.bfloat16

# rope-pair permutation: old pair (i, i+64) -> within-quadrant pair
# (32q+j, 32q+16+j) with q=i//16, j=i%16, so one stream_shuffle mask
# (swap 16-partition halves of each 32-partition quadrant) does the
# rotate-half partition move on the DVE.
_DPERM = np.empty(D, dtype=np.int64)          # old index of each new slot
for _q in range(4):
    for _j in range(16):
        _DPERM[32 * _q + _j] = 16 * _q + _j
        _DPERM[32 * _q + 16 + _j] = 64 + 16 * _q + _j
SHUF_MASK = list(range(16, 32)) + list(range(0, 16))

WARM = 56                 # PE warmup matmuls (ones[P,P] @ ones[P,P])


def _host_consts():
    inv_freq = 1.0 / (THETA ** (np.arange(0, D, 2, dtype=np.float64) / D))
    freqs = np.outer(np.arange(T, dtype=np.float64), inv_freq)    # [T, 64]
    emb = np.concatenate([freqs, freqs], axis=-1)                 # [T, 128]
    cos = np.cos(emb).T                                           # [128, T]
    sin = np.sin(emb).T
    # rotate_half sign folded into sin: out = x*cos + shuffle(x)*sin_signed
    sin_signed = np.concatenate([-sin[:D // 2], sin[D // 2:]], axis=0)
    # transposed-S diagonal-block multiplicative mask: rows tk, cols tq;
    # valid iff tq >= tk
    mask_t = np.triu(np.ones((P, P), np.float32)).astype(BF)
    return {
        "cos_t": np.ascontiguousarray(cos[_DPERM]).astype(BF),
        "sin_t": np.ascontiguousarray(sin_signed[_DPERM]).astype(BF),
        "mask_t": mask_t,
    }


def _build(nc):
    hid = nc.dram_tensor("hidden_pk", [P, KT_HID, TOK], BF16,
                         kind="ExternalInput").ap()
    wq = nc.dram_tensor("wq_pk", [P, KT_HID, NH * D], BF16,
                        kind="ExternalInput").ap()
    wk = nc.dram_tensor("wk_g", [NKV, P, KT_HID, D], BF16,
                        kind="ExternalInput").ap()
    wv = nc.dram_tensor("wv_pk", [P, KT_HID, NKV * D], BF16,
                        kind="ExternalInput").ap()
    wo = nc.dram_tensor("wo_pk", [P, KT_HID, HID], BF16,
                        kind="ExternalInput").ap()
    cos_t = nc.dram_tensor("cos_t", [P, T], BF16, kind="ExternalInput").ap()
    sin_t = nc.dram_tensor("sin_t", [P, T], BF16, kind="ExternalInput").ap()
    mask_t = nc.dram_tensor("mask_t", [P, P], BF16, kind="ExternalInput").ap()
    out = nc.dram_tensor("out", [TOK, HID], BF16, kind="ExternalOutput").ap()

    with tile.TileContext(nc) as tc, ExitStack() as ctx:
        # ---- pools with cross-phase lifetimes ----
        consts = ctx.enter_context(tc.tile_pool(name="consts", bufs=1))

        cos_sb = consts.tile([P, T], BF16, tag="cos")
        sin_sb = consts.tile([P, T], BF16, tag="sin")
        maskt_sb = consts.tile([P, P], BF16, tag="maskt")
        ones_bf = consts.tile([P, P], BF16, tag="ones")
        # gpsimd comes up first (~6.1us) -> warmup deps ready earliest
        nc.gpsimd.memset(ones_bf, 1.0)

        qkvpool = ctx.enter_context(tc.tile_pool(name="qkv", bufs=1))
        qt_sb = qkvpool.tile([P, NH, BL, T], BF16, tag="qt")    # [d,h,b,t]
        kt_sb = qkvpool.tile([P, NKV, BL, T], BF16, tag="kt")   # [d,g,b,t]
        v_sb = qkvpool.tile([P, NTOK_T, NKV * D], BF16, tag="v")
        otpool = ctx.enter_context(tc.tile_pool(name="otpool", bufs=1))
        ot_sb = otpool.tile([P, NH, BL, T], BF16, tag="ot")     # [d,h,b,t]

        # single PSUM pool for BOTH phases: phase A draws its K-wave/V-proj
        # psums from the same tag rotations phase B uses, so there is no
        # psum pool close/open barrier between the phases
        psAll = ctx.enter_context(
            tc.tile_pool(name="psAll", bufs=1, space=bass.MemorySpace.PSUM))

        wpool = ctx.enter_context(tc.tile_pool(name="wpool", bufs=1))
        wq_sb = wpool.tile([P, KT_HID, NH * D], BF16, tag="wq")
        wk_sb = wpool.tile([P, NKV, KT_HID, D], BF16, tag="wk")
        wv_sb = wpool.tile([P, KT_HID, NKV * D], BF16, tag="wv")
        wo_sb = wpool.tile([P, KT_HID, HID], BF16, tag="wo")
        xt_sb = wpool.tile([P, KT_HID, TOK], BF16, tag="xt")    # [hid,k,tok]

        # ---- input loads: first-use order, aware that the DMA engines
        # alternate between the two rings' FIFOs (global completion order
        # ~= zip of the two lists) and that a completed transfer is only
        # CONSUMABLE ~2.5us later (completion->semaphore->engine latency).
        # Wave 1 of the K proj needs wk0+wk1+xt0 first; wk2/wk3 are only
        # needed when wave 2 starts (~7us later) ----
        nc.sync.dma_start(out=wk_sb[:, 0], in_=wk[0])
        nc.scalar.dma_start(out=wk_sb[:, 1], in_=wk[1])
        for k in range(KT_HID):
            eng = nc.sync if k % 2 == 0 else nc.scalar
            eng.dma_start(out=xt_sb[:, k, :], in_=hid[:, k, :])
        nc.sync.dma_start(out=wk_sb[:, 2], in_=wk[2])
        nc.scalar.dma_start(out=wk_sb[:, 3], in_=wk[3])
        nc.sync.dma_start(out=cos_sb, in_=cos_t)
        nc.scalar.dma_start(out=sin_sb, in_=sin_t)
        nc.scalar.dma_start(out=wv_sb, in_=wv)
        nc.sync.dma_start(out=wq_sb[:, 0:4, :], in_=wq[:, 0:4, :])
        nc.scalar.dma_start(out=wq_sb[:, 4:8, :], in_=wq[:, 4:8, :])
        nc.sync.dma_start(out=maskt_sb, in_=mask_t)
        nc.sync.dma_start(out=wo_sb[:, 0:4, :], in_=wo[:, 0:4, :])
        nc.scalar.dma_start(out=wo_sb[:, 4:8, :], in_=wo[:, 4:8, :])

        cos_bc = cos_sb[:, None, :].to_broadcast([P, BL, T])
        sin_bc = sin_sb[:, None, :].to_broadcast([P, BL, T])

        def _rope_dve(raw, out_sl, tmp_pool):
            """Full rope on the DVE: partition-half swap via stream_shuffle
            (head-dim permuted so pairs sit within 32-partition quadrants),
            then out = raw*cos + shuffled*sin_signed."""
            swp = tmp_pool.tile([P, BL, T], BF16, tag="rope_swp", bufs=2,
                                name="swp")
            nc.vector.stream_shuffle(swp, raw, SHUF_MASK)
            tmp = tmp_pool.tile([P, BL, T], BF16, tag="rope_tmp", bufs=2,
                                name="tmp")
            nc.vector.tensor_mul(tmp, swp, sin_bc)
            nc.vector.tensor_mul(out_sl, raw, cos_bc)
            nc.vector.tensor_add(out_sl, out_sl, tmp)

        # rotation over the phase-B psum tags for phase A's draws; at most
        # 4 buffers are live at once (one K wave), and the 7-slot cycle
        # guarantees a buffer is never re-drawn while still live
        from itertools import cycle
        _acycle = cycle([("qps", 2), ("qps", 2), ("sps", 3), ("sps", 3),
                         ("sps", 3), ("ops", 2), ("ops", 2)])

        def _psA_tile():
            tag, bufs = next(_acycle)
            return psAll.tile([P, T], FP32, tag=tag, bufs=bufs, name="aps")

        # ---- phase A: warmup + K proj + V proj ----
        with ExitStack() as phase1:
            ropet = phase1.enter_context(tc.tile_pool(name="ropetA", bufs=2))

            # PE warmup: dependency-free matmuls so the PE queue ramps the
            # HAM clock gate (1.2 -> 2.4 GHz) while Wk + the first X planes
            # are in flight; sized to end right as they land (~11.5us)
            wps = _psA_tile()
            for w in range(WARM):
                nc.tensor.matmul(wps[:, 0:P], ones_bf, ones_bf,
                                 start=True, stop=True, skip_group_check=True)

            # KT = Wk.T @ XT: the 8 (g, c) chains run as two 4-chain waves
            # interleaved plane-by-plane so the PE chases the X DMA
            # wavefront (plane k feeds 4 matmuls ~0.85us while the next
            # plane needs ~0.7us to load); wave 1 only needs wk0/wk1 so it
            # can start as soon as xt0 is consumable
            kraws = [ropet.tile([P, BL, T], BF16, tag="rope_raw", bufs=4,
                                name="kraw") for _ in range(NKV)]
            for wave in range(2):
                gs = (0, 1) if wave == 0 else (2, 3)
                kps = {(g, c): _psA_tile()
                       for g in gs for c in range(BL)}
                for k in range(KT_HID):
                    for (g, c), ps in kps.items():
                        nc.tensor.matmul(
                            ps,
                            wk_sb[:, g, k, :],
                            xt_sb[:, k, c * T:(c + 1) * T],
                            start=(k == 0), stop=(k == KT_HID - 1),
                            skip_group_check=True)
                for g in gs:
                    for c in range(BL):
                        nc.scalar.copy(kraws[g][:, c, :], kps[(g, c)])
                    _rope_dve(kraws[g], kt_sb[:, g], ropet)
            # V natural: [tok, dkv]
            for tt in range(NTOK_T):
                ps = _psA_tile()
                for k in range(KT_HID):
                    nc.tensor.matmul(
                        ps[:, :NKV * D],
                        xt_sb[:, k, tt * P:(tt + 1) * P],
                        wv_sb[:, k, :],
                        start=(k == 0), stop=(k == KT_HID - 1))
                nc.scalar.copy(v_sb[:, tt, :], ps[:, :NKV * D])

        # ---- phase B: Q proj interleaved with attention ----
        with ExitStack() as phase2:
            ropet = phase2.enter_context(tc.tile_pool(name="ropetB", bufs=2))
            ptpool = phase2.enter_context(tc.tile_pool(name="ptpool", bufs=5))
            stats = phase2.enter_context(tc.tile_pool(name="stats", bufs=3))
            psM = psAll

            pend = deque()
            head_state = {}
            qraw_state = {}
            rope_pending = deque()

            def emit_qchain(h, c):
                ps = psM.tile([P, T], FP32, tag="qps", bufs=2, name="qps")
                for k in range(KT_HID):
                    nc.tensor.matmul(
                        ps,
                        wq_sb[:, k, h * P:(h + 1) * P],
                        xt_sb[:, k, c * T:(c + 1) * T],
                        start=(k == 0), stop=(k == KT_HID - 1))
                if c == 0:
                    qraw_state[h] = ropet.tile([P, BL, T], BF16,
                                               tag="rope_raw", bufs=2,
                                               name="qraw")
                qraw = qraw_state[h]
                nc.scalar.copy(qraw[:, c, :], ps)
                if c == BL - 1:
                    rope_pending.append((qraw, h))
                    del qraw_state[h]

            def emit_rope_dve():
                qraw, h = rope_pending.popleft()
                _rope_dve(qraw, qt_sb[:, h], ropet)

            def emit_item_S(b, h):
                """S matmuls + exps for all 4 tk-blocks of one (batch, head),
                then ONE merged causal-mask multiply: each PT_j plane stores
                columns [lo:T] shifted to plane-local [0:T-lo], so all four
                diagonal blocks line up at plane-local [0:P] and mask in a
                single strided DVE op."""
                g = h // GROUPS
                pt = ptpool.tile([P, NTQ, T], BF16, tag="pt", bufs=5,
                                 name="pt")
                for j in range(NTQ):
                    lo = j * P
                    st_ps = psM.tile([P, T], FP32, tag="sps", bufs=3,
                                     name="sps")
                    nc.tensor.matmul(
                        st_ps[:, lo:T],
                        kt_sb[:, g, b, lo:lo + P],
                        qt_sb[:, h, b, lo:T],
                        start=True, stop=True)
                    # exp -> PT_j, already transposed for the PV matmul;
                    # 1/sqrt(D) folded into the activation scale
                    # (no row-max: logits are O(1) by construction)
                    nc.scalar.activation(
                        out=pt[:, j, 0:T - lo], in_=st_ps[:, lo:T],
                        func=mybir.ActivationFunctionType.Exp,
                        bias=0.0, scale=SCALE)
                nc.vector.tensor_mul(
                    pt[:, :, 0:P], pt[:, :, 0:P],
                    maskt_sb[:, None, :].to_broadcast([P, NTQ, P]))
                for j in range(NTQ):
                    pend.append((b, h, j, pt))

            def drain_one():
                b, h, j, pt = pend.popleft()
                g = h // GROUPS
                lo = j * P
                st = head_state.get((b, h))
                if st is None:
                    o_ps_new = psM.tile([P, T], FP32, tag="ops", bufs=2,
                                        name="ops")
                    cs_ps_new = psM.tile([P, T], FP32, tag="cps", bufs=1,
                                         name="cps")
                    st = head_state[(b, h)] = (o_ps_new, cs_ps_new)
                o_ps, cs_ps = st
                # colsum with an ALL-ONES stationary: psum rows all carry
                # the denominator (partition-broadcast for free);
                # OT += V_j.T @ PT_j chained in the other psum
                nc.tensor.matmul(
                    cs_ps[:, lo:T] if j else cs_ps[:, :],
                    ones_bf,
                    pt[:, j, 0:T - lo],
                    start=(j == 0), stop=(j == NTQ - 1),
                    skip_group_check=True)
                nc.tensor.matmul(
                    o_ps[:, lo:T] if j else o_ps[:, :],
                    v_sb[:, b * NTQ + j, g * D:(g + 1) * D],
                    pt[:, j, 0:T - lo],
                    start=(j == 0), stop=(j == NTQ - 1),
                    skip_group_check=True)
                if j == NTQ - 1:
                    # per-head softmax normalization, fully on the DVE: the
                    # all-ones colsum already broadcast the denominator
                    # across partitions, so reciprocal + one multiply
                    rb = stats.tile([P, T], FP32, tag="rb", bufs=2,
                                    name="rb")
                    nc.vector.reciprocal_approx_fast(rb, cs_ps)
                    nc.vector.tensor_mul(ot_sb[:, h, b], o_ps, rb)
                    del head_state[(b, h)]

            def emit_att(b, h):
                emit_item_S(b, h)
                while len(pend) > 11:
                    drain_one()

            def emit_oproj(tt, cchunk):
                # one out-projection chunk: out[tt-block, chunk] as its own
                # k-chain, sharing the qps psum rotation with the (finished)
                # Q chains so it can interleave with the attention tail
                ps = psM.tile([P, T], FP32, tag="qps", bufs=2, name="opps")
                for k in range(KT_HID):
                    nc.tensor.matmul(
                        ps,
                        ot_sb[:, k, tt // NTQ,
                              (tt % NTQ) * P:(tt % NTQ + 1) * P],
                        wo_sb[:, k, cchunk * T:(cchunk + 1) * T],
                        start=(k == 0), stop=(k == KT_HID - 1))
                o_tile = stats.tile([P, T], BF16, tag="oout", bufs=3,
                                    name="o_tile")
                if tt == NTOK_T - 1 and cchunk == 1:
                    # very last chunk: drain as TWO halves, one per copy
                    # engine, running CONCURRENTLY -- psum-read copies are
                    # latency-bound (~0.6us effective each regardless of
                    # width), so two wide copies in parallel beat four
                    # serialized quarters; copies emitted before stores so
                    # no dma_start (which occupies its issuing engine ~1us)
                    # delays a copy
                    HT = T // 2
                    base = cchunk * T
                    nc.vector.tensor_copy(o_tile[:, 0:HT], ps[:, 0:HT])
                    nc.scalar.copy(o_tile[:, HT:T], ps[:, HT:T])
                    nc.sync.dma_start(
                        out=out[tt * P:(tt + 1) * P, base:base + HT],
                        in_=o_tile[:, 0:HT])
                    nc.scalar.dma_start(
                        out=out[tt * P:(tt + 1) * P, base + HT:base + T],
                        in_=o_tile[:, HT:T])
                    return
                # copies always on ACT: a PE-dependent copy on the DVE would
                # head-of-line delay the recips/norms that gate later chains;
                # stores always on the otherwise-idle sync queue
                nc.scalar.copy(o_tile, ps)
                nc.sync.dma_start(
                    out=out[tt * P:(tt + 1) * P,
                            cchunk * T:(cchunk + 1) * T],
                    in_=o_tile)

            # attention lags the Q chains by 3 heads and the DVE part of
            # each rope lags its chain by 1 slot: every engine-queue entry
            # has its dependencies resolved before it reaches the queue
            # head, so the strict-FIFO DVE queue never head-of-line blocks
            # the causal masks that gate the PE's colsum/PV matmuls
            LAG = 3
            for h in range(NH):
                if h >= LAG:
                    emit_att(0, h - LAG)
                emit_qchain(h, 0)
                if h >= LAG:
                    emit_att(1, h - LAG)
                if rope_pending and h >= 1:
                    emit_rope_dve()
                emit_qchain(h, 1)
            while rope_pending:
                emit_rope_dve()
            # tail: batch-0 attention first, then interleave out-projection
            # chunks (batch-0 token tiles first) with the remaining
            # ACT/DVE-bound attention so the PE stays dense
            for h in range(NH - LAG, NH):
                emit_att(0, h)
            emit_att(1, NH - LAG)
            while len(pend) > 4:
                drain_one()              # flush: norms (0, *) all emitted
            emit_oproj(0, 0)
            emit_oproj(0, 1)
            emit_att(1, NH - 2)
            emit_oproj(1, 0)
            emit_oproj(1, 1)
            emit_att(1, NH - 1)
            emit_oproj(2, 0)
            emit_oproj(2, 1)
            while len(pend) > 4:
                drain_one()              # flush: norm (1, NH-2) emitted
            emit_oproj(3, 0)
            while pend:
                drain_one()              # norm (1, NH-1)
            emit_oproj(3, 1)
            for tt in range(4, NTOK_T):
                emit_oproj(tt, 0)
                emit_oproj(tt, 1)


def _prep_weights(Wq, Wk, Wv, Wo):
    """Host-side: bf16, rope-pair permutation of the q/k head dims, and
    rearrangement to the sbuf fill layout [p, k, n] (contiguous per
    partition) so load DMA descriptors are large."""
    wq = np.asarray(Wq, dtype=np.float32).astype(BF)
    wk = np.asarray(Wk, dtype=np.float32).astype(BF)
    wv = np.asarray(Wv, dtype=np.float32).astype(BF)
    wo = np.asarray(Wo, dtype=np.float32).astype(BF)
    # permute the per-head d axis of Wq/Wk
    wq = wq.reshape(HID, NH, D)[:, :, _DPERM].reshape(HID, NH * D)
    wk = wk.reshape(HID, NKV, D)[:, :, _DPERM].reshape(HID, NKV * D)
    wq_pk = np.ascontiguousarray(
        wq.reshape(KT_HID, P, NH * D).transpose(1, 0, 2))
    wk_g = np.ascontiguousarray(
        wk.reshape(KT_HID, P, NKV, D).transpose(2, 1, 0, 3))
    wv_pk = np.ascontiguousarray(
        wv.reshape(KT_HID, P, NKV * D).transpose(1, 0, 2))
    wo_pk = np.ascontiguousarray(
        wo.reshape(KT_HID, P, HID).transpose(1, 0, 2))
    return {"wq_pk": wq_pk, "wk_g": wk_g, "wv_pk": wv_pk, "wo_pk": wo_pk}


_COMPILED = None


def _get_compiled():
    global _COMPILED
    if _COMPILED is None:
        nc = bacc.Bacc("TRN2", target_bir_lowering=False, debug=False)
        _build(nc)
        nc.compile()
        _COMPILED = nc
    return _COMPILED


def kernel(hidden_states, Wq, Wk, Wv, Wo, _trace=False, _trace_kwargs=None):
    hs = np.asarray(hidden_states, dtype=np.float32).astype(BF)
    weights = _prep_weights(Wq, Wk, Wv, Wo)
    consts = _host_consts()
    nc = _get_compiled()
    in_maps = []
    for c in range(NCORES):
        # ship X pre-transposed AND pre-tiled ([p, k, tok]) so the kernel's
        # loads are plane-sliced contiguous DMAs
        shard = hs[BL * c: BL * (c + 1)].reshape(TOK, HID).T  # [HID, TOK]
        shard_pk = np.ascontiguousarray(
            shard.reshape(KT_HID, P, TOK).transpose(1, 0, 2))
        in_maps.append({"hidden_pk": shard_pk, **weights, **consts})
    res = run_bass_kernel_spmd(
        nc, in_maps, list(range(NCORES)), trace=_trace,
        **(_trace_kwargs or {}))
    outs = [np.asarray(r["out"]).astype(np.float32).reshape(BL, T, HID)
            for r in res.results]
    full = np.concatenate(outs, axis=0)
    if _trace:
        return full, res
    return full


# revision 37
# speedup vs baseline: 1.0343x; 1.0012x over previous
"""GQA attention kernel for Trainium2, data-parallel over batch on 8 NeuronCores.

Per-core problem (2 of 16 batches): X [1024tok, 1024] -> QKV proj -> RoPE ->
causal GQA attention (8 q heads, 4 kv heads, D=128) -> out proj [1024, 1024].

v3 layout strategy (PE-occupancy-driven; baseline v2 was ~146.4us):
  - All dram tensors are host-side pre-arranged to the exact sbuf fill
    layout ([p, k, n]) so every load DMA runs with 4-16KB contiguous
    descriptors, and loads are ordered by first use with the K-projection
    dependencies (X, then per-g Wk chunks) first across both HWDGE rings.
  - RoPE's partition-half swap is done ON THE DVE via stream_shuffle: the
    head dim of Wq/Wk (and cos/sin rows) is permuted so each rotate-half
    pair (i, i+64) lands 16 partitions apart inside one 32-partition
    quadrant (S = q.k is invariant under a consistent d-permutation).
    This removes all sbuf<->sbuf swap DMAs (3MB of ring traffic that used
    to compete with the weight loads) and makes rope a pure DVE chain.
  - The 1/sqrt(D) scale is folded into the exp's activation scale, so one
    UNSCALED cos/sin table pair [128, 512] is shared by Q and K rope and
    broadcast over the two batch chunks (0.25MB loaded vs 1MB).
  - Softmax denominator: the colsum matmul uses an ALL-ONES [128,128]
    stationary operand, so the psum result is the denominator already
    broadcast across partitions (same PE streaming cost, cheaper
    instruction shape than M=1) and normalization is a DVE
    reciprocal_approx_fast + one multiply -- no GPSIMD
    partition_broadcast; GPSIMD retires from the kernel entirely.
    (A single tensor_tensor divide would be cheaper still but the BIR
    verifier rejects divide on the DVE.)
  - PE warmup is dependency-light (ones@ones after a gpsimd memset) so it
    starts as soon as the PE queue comes up (~6.6us) and is sized to end
    exactly when X+Wk land (~15us), covering the whole load latency.
  - Output dram tensor is bf16 (host upcasts): halves store traffic; all
    stores issue from the idle sync queue (a dma_start occupies its
    issuing engine ~1us), and the final chunk drains in [128,128]
    quarters, copies (DVE/ACT alternating) emitted before any store.
  - Engine assignment is FIFO-queue-driven (strict in-order queues):
      ACT:    exps, psum->sbuf copies (qraw/V/out tiles)
      DVE:    rope (shuffle+mul+mul+add), merged causal masks, divides
      GPSIMD: nothing (only the startup ones memset)
      sync/scalar DMA rings: loads first-use-ordered, then output stores
  - Same software pipeline as v2: S/exp emission runs a full head ahead of
    the colsum/PV consumers (deque, drain threshold 11), attention
    consumption lags Q-chain emission by 3 heads, rope DVE chains are
    emitted one head-slot late.
"""

import numpy as np
import ml_dtypes
from collections import deque
from contextlib import ExitStack

import concourse.bass as bass
import concourse.tile as tile
from concourse import bacc, mybir
from concourse.bass_utils import run_bass_kernel_spmd

B, T, HID = 16, 512, 1024
NH, NKV, D = 8, 4, 128
THETA = 10000.0
NCORES = 8
BL = B // NCORES          # local batches per core
TOK = BL * T              # local tokens
P = 128
KT_HID = HID // P         # 8 contraction tiles over hidden
NTQ = T // P              # 4 tk/tq tiles per sequence
NTOK_T = TOK // P         # 8 token tiles per core
GROUPS = NH // NKV        # 2 q heads per kv head
SCALE = 1.0 / float(np.sqrt(D))
FP32 = mybir.dt.float32
BF16 = mybir.dt.bfloat16
BF = ml_dtypes.bfloat16

# rope-pair permutation: old pair (i, i+64) -> within-quadrant pair
# (32q+j, 32q+16+j) with q=i//16, j=i%16, so one stream_shuffle mask
# (swap 16-partition halves of each 32-partition quadrant) does the
# rotate-half partition move on the DVE.
_DPERM = np.empty(D, dtype=np.int64)          # old index of each new slot
for _q in range(4):
    for _j in range(16):
        _DPERM[32 * _q + _j] = 16 * _q + _j
        _DPERM[32 * _q + 16 + _j] = 64 + 16 * _q + _j
SHUF_MASK = list(range(16, 32)) + list(range(0, 16))

WARM = 56                 # PE warmup matmuls (ones[P,P] @ ones[P,P])


def _host_consts():
    inv_freq = 1.0 / (THETA ** (np.arange(0, D, 2, dtype=np.float64) / D))
    freqs = np.outer(np.arange(T, dtype=np.float64), inv_freq)    # [T, 64]
    emb = np.concatenate([freqs, freqs], axis=-1)                 # [T, 128]
    cos = np.cos(emb).T                                           # [128, T]
    sin = np.sin(emb).T
    # rotate_half sign folded into sin: out = x*cos + shuffle(x)*sin_signed
    sin_signed = np.concatenate([-sin[:D // 2], sin[D // 2:]], axis=0)
    # transposed-S diagonal-block multiplicative mask: rows tk, cols tq;
    # valid iff tq >= tk
    mask_t = np.triu(np.ones((P, P), np.float32)).astype(BF)
    return {
        "cos_t": np.ascontiguousarray(cos[_DPERM]).astype(BF),
        "sin_t": np.ascontiguousarray(sin_signed[_DPERM]).astype(BF),
        "mask_t": mask_t,
    }


def _build(nc):
    hid = nc.dram_tensor("hidden_pk", [P, KT_HID, TOK], BF16,
                         kind="ExternalInput").ap()
    wq = nc.dram_tensor("wq_pk", [P, KT_HID, NH * D], BF16,
                        kind="ExternalInput").ap()
    wk = nc.dram_tensor("wk_g", [NKV, P, KT_HID, D], BF16,
                        kind="ExternalInput").ap()
    wv = nc.dram_tensor("wv_pk", [P, KT_HID, NKV * D], BF16,
                        kind="ExternalInput").ap()
    wo = nc.dram_tensor("wo_pk", [P, KT_HID, HID], BF16,
                        kind="ExternalInput").ap()
    cos_t = nc.dram_tensor("cos_t", [P, T], BF16, kind="ExternalInput").ap()
    sin_t = nc.dram_tensor("sin_t", [P, T], BF16, kind="ExternalInput").ap()
    mask_t = nc.dram_tensor("mask_t", [P, P], BF16, kind="ExternalInput").ap()
    out = nc.dram_tensor("out", [TOK, HID], BF16, kind="ExternalOutput").ap()

    with tile.TileContext(nc) as tc, ExitStack() as ctx:
        # ---- pools with cross-phase lifetimes ----
        consts = ctx.enter_context(tc.tile_pool(name="consts", bufs=1))

        cos_sb = consts.tile([P, T], BF16, tag="cos")
        sin_sb = consts.tile([P, T], BF16, tag="sin")
        maskt_sb = consts.tile([P, P], BF16, tag="maskt")
        ones_bf = consts.tile([P, P], BF16, tag="ones")
        # gpsimd comes up first (~6.1us) -> warmup deps ready earliest
        nc.gpsimd.memset(ones_bf, 1.0)

        qkvpool = ctx.enter_context(tc.tile_pool(name="qkv", bufs=1))
        qt_sb = qkvpool.tile([P, NH, BL, T], BF16, tag="qt")    # [d,h,b,t]
        kt_sb = qkvpool.tile([P, NKV, BL, T], BF16, tag="kt")   # [d,g,b,t]
        v_sb = qkvpool.tile([P, NTOK_T, NKV * D], BF16, tag="v")
        otpool = ctx.enter_context(tc.tile_pool(name="otpool", bufs=1))
        ot_sb = otpool.tile([P, NH, BL, T], BF16, tag="ot")     # [d,h,b,t]

        # single PSUM pool for BOTH phases: phase A draws its K-wave/V-proj
        # psums from the same tag rotations phase B uses, so there is no
        # psum pool close/open barrier between the phases
        psAll = ctx.enter_context(
            tc.tile_pool(name="psAll", bufs=1, space=bass.MemorySpace.PSUM))

        wpool = ctx.enter_context(tc.tile_pool(name="wpool", bufs=1))
        wq_sb = wpool.tile([P, KT_HID, NH * D], BF16, tag="wq")
        wk_sb = wpool.tile([P, NKV, KT_HID, D], BF16, tag="wk")
        wv_sb = wpool.tile([P, KT_HID, NKV * D], BF16, tag="wv")
        wo_sb = wpool.tile([P, KT_HID, HID], BF16, tag="wo")
        xt_sb = wpool.tile([P, KT_HID, TOK], BF16, tag="xt")    # [hid,k,tok]

        # ---- input loads: first-use order, aware that the DMA engines
        # alternate between the two rings' FIFOs (global completion order
        # ~= zip of the two lists) and that a completed transfer is only
        # CONSUMABLE ~2.5us later (completion->semaphore->engine latency).
        # Wave 1 of the K proj needs wk0+wk1+xt0 first; wk2/wk3 are only
        # needed when wave 2 starts (~7us later) ----
        nc.sync.dma_start(out=wk_sb[:, 0], in_=wk[0])
        nc.scalar.dma_start(out=wk_sb[:, 1], in_=wk[1])
        for k in range(KT_HID):
            eng = nc.sync if k % 2 == 0 else nc.scalar
            eng.dma_start(out=xt_sb[:, k, :], in_=hid[:, k, :])
        nc.sync.dma_start(out=wk_sb[:, 2], in_=wk[2])
        nc.scalar.dma_start(out=wk_sb[:, 3], in_=wk[3])
        nc.sync.dma_start(out=cos_sb, in_=cos_t)
        nc.scalar.dma_start(out=sin_sb, in_=sin_t)
        nc.scalar.dma_start(out=wv_sb, in_=wv)
        nc.sync.dma_start(out=wq_sb[:, 0:4, :], in_=wq[:, 0:4, :])
        nc.scalar.dma_start(out=wq_sb[:, 4:8, :], in_=wq[:, 4:8, :])
        nc.sync.dma_start(out=maskt_sb, in_=mask_t)
        nc.sync.dma_start(out=wo_sb[:, 0:4, :], in_=wo[:, 0:4, :])
        nc.scalar.dma_start(out=wo_sb[:, 4:8, :], in_=wo[:, 4:8, :])

        cos_bc = cos_sb[:, None, :].to_broadcast([P, BL, T])
        sin_bc = sin_sb[:, None, :].to_broadcast([P, BL, T])

        def _rope_dve(raw, out_sl, tmp_pool):
            """Full rope on the DVE: partition-half swap via stream_shuffle
            (head-dim permuted so pairs sit within 32-partition quadrants),
            then out = raw*cos + shuffled*sin_signed."""
            swp = tmp_pool.tile([P, BL, T], BF16, tag="rope_swp", bufs=2,
                                name="swp")
            nc.vector.stream_shuffle(swp, raw, SHUF_MASK)
            tmp = tmp_pool.tile([P, BL, T], BF16, tag="rope_tmp", bufs=2,
                                name="tmp")
            nc.vector.tensor_mul(tmp, swp, sin_bc)
            nc.vector.tensor_mul(out_sl, raw, cos_bc)
            nc.vector.tensor_add(out_sl, out_sl, tmp)

        # rotation over the phase-B psum tags for phase A's draws; at most
        # 4 buffers are live at once (one K wave), and the 7-slot cycle
        # guarantees a buffer is never re-drawn while still live
        from itertools import cycle
        _acycle = cycle([("qps", 2), ("qps", 2), ("sps", 3), ("sps", 3),
                         ("sps", 3), ("ops", 2), ("ops", 2)])

        def _psA_tile():
            tag, bufs = next(_acycle)
            return psAll.tile([P, T], FP32, tag=tag, bufs=bufs, name="aps")

        # ---- phase A: warmup + K proj + V proj ----
        with ExitStack() as phase1:
            ropet = phase1.enter_context(tc.tile_pool(name="ropetA", bufs=2))

            # PE warmup: dependency-free matmuls so the PE queue ramps the
            # HAM clock gate (1.2 -> 2.4 GHz) while Wk + the first X planes
            # are in flight; sized to end right as they land (~11.5us)
            wps = _psA_tile()
            for w in range(WARM):
                nc.tensor.matmul(wps[:, 0:P], ones_bf, ones_bf,
                                 start=True, stop=True, skip_group_check=True)

            # KT = Wk.T @ XT: the 8 (g, c) chains run as two 4-chain waves
            # interleaved plane-by-plane so the PE chases the X DMA
            # wavefront (plane k feeds 4 matmuls ~0.85us while the next
            # plane needs ~0.7us to load); wave 1 only needs wk0/wk1 so it
            # can start as soon as xt0 is consumable
            kraws = [ropet.tile([P, BL, T], BF16, tag="rope_raw", bufs=4,
                                name="kraw") for _ in range(NKV)]
            for wave in range(2):
                gs = (0, 1) if wave == 0 else (2, 3)
                kps = {(g, c): _psA_tile()
                       for g in gs for c in range(BL)}
                for k in range(KT_HID):
                    for (g, c), ps in kps.items():
                        nc.tensor.matmul(
                            ps,
                            wk_sb[:, g, k, :],
                            xt_sb[:, k, c * T:(c + 1) * T],
                            start=(k == 0), stop=(k == KT_HID - 1),
                            skip_group_check=True)
                for g in gs:
                    for c in range(BL):
                        nc.scalar.copy(kraws[g][:, c, :], kps[(g, c)])
                    _rope_dve(kraws[g], kt_sb[:, g], ropet)
            # V natural: [tok, dkv]
            for tt in range(NTOK_T):
                ps = _psA_tile()
                for k in range(KT_HID):
                    nc.tensor.matmul(
                        ps[:, :NKV * D],
                        xt_sb[:, k, tt * P:(tt + 1) * P],
                        wv_sb[:, k, :],
                        start=(k == 0), stop=(k == KT_HID - 1))
                nc.scalar.copy(v_sb[:, tt, :], ps[:, :NKV * D])

        # ---- phase B: Q proj interleaved with attention ----
        with ExitStack() as phase2:
            ropet = phase2.enter_context(tc.tile_pool(name="ropetB", bufs=2))
            ptpool = phase2.enter_context(tc.tile_pool(name="ptpool", bufs=5))
            stats = phase2.enter_context(tc.tile_pool(name="stats", bufs=3))
            psM = psAll

            pend = deque()
            head_state = {}
            qraw_state = {}
            rope_pending = deque()

            def emit_qchain(h, c):
                ps = psM.tile([P, T], FP32, tag="qps", bufs=2, name="qps")
                for k in range(KT_HID):
                    nc.tensor.matmul(
                        ps,
                        wq_sb[:, k, h * P:(h + 1) * P],
                        xt_sb[:, k, c * T:(c + 1) * T],
                        start=(k == 0), stop=(k == KT_HID - 1))
                if c == 0:
                    qraw_state[h] = ropet.tile([P, BL, T], BF16,
                                               tag="rope_raw", bufs=2,
                                               name="qraw")
                qraw = qraw_state[h]
                nc.scalar.copy(qraw[:, c, :], ps)
                if c == BL - 1:
                    rope_pending.append((qraw, h))
                    del qraw_state[h]

            def emit_rope_dve():
                qraw, h = rope_pending.popleft()
                _rope_dve(qraw, qt_sb[:, h], ropet)

            def emit_item_S(b, h):
                """S matmuls + exps for all 4 tk-blocks of one (batch, head),
                then ONE merged causal-mask multiply: each PT_j plane stores
                columns [lo:T] shifted to plane-local [0:T-lo], so all four
                diagonal blocks line up at plane-local [0:P] and mask in a
                single strided DVE op."""
                g = h // GROUPS
                pt = ptpool.tile([P, NTQ, T], BF16, tag="pt", bufs=5,
                                 name="pt")
                for j in range(NTQ):
                    lo = j * P
                    st_ps = psM.tile([P, T], FP32, tag="sps", bufs=3,
                                     name="sps")
                    nc.tensor.matmul(
                        st_ps[:, lo:T],
                        kt_sb[:, g, b, lo:lo + P],
                        qt_sb[:, h, b, lo:T],
                        start=True, stop=True)
                    # exp -> PT_j, already transposed for the PV matmul;
                    # 1/sqrt(D) folded into the activation scale
                    # (no row-max: logits are O(1) by construction)
                    nc.scalar.activation(
                        out=pt[:, j, 0:T - lo], in_=st_ps[:, lo:T],
                        func=mybir.ActivationFunctionType.Exp,
                        bias=0.0, scale=SCALE)
                nc.vector.tensor_mul(
                    pt[:, :, 0:P], pt[:, :, 0:P],
                    maskt_sb[:, None, :].to_broadcast([P, NTQ, P]))
                for j in range(NTQ):
                    pend.append((b, h, j, pt))

            def drain_one():
                b, h, j, pt = pend.popleft()
                g = h // GROUPS
                lo = j * P
                st = head_state.get((b, h))
                if st is None:
                    o_ps_new = psM.tile([P, T], FP32, tag="ops", bufs=2,
                                        name="ops")
                    cs_ps_new = psM.tile([P, T], FP32, tag="cps", bufs=1,
                                         name="cps")
                    st = head_state[(b, h)] = (o_ps_new, cs_ps_new)
                o_ps, cs_ps = st
                # colsum with an ALL-ONES stationary: psum rows all carry
                # the denominator (partition-broadcast for free);
                # OT += V_j.T @ PT_j chained in the other psum
                nc.tensor.matmul(
                    cs_ps[:, lo:T] if j else cs_ps[:, :],
                    ones_bf,
                    pt[:, j, 0:T - lo],
                    start=(j == 0), stop=(j == NTQ - 1),
                    skip_group_check=True)
                nc.tensor.matmul(
                    o_ps[:, lo:T] if j else o_ps[:, :],
                    v_sb[:, b * NTQ + j, g * D:(g + 1) * D],
                    pt[:, j, 0:T - lo],
                    start=(j == 0), stop=(j == NTQ - 1),
                    skip_group_check=True)
                if j == NTQ - 1:
                    # per-head softmax normalization, fully on the DVE: the
                    # all-ones colsum already broadcast the denominator
                    # across partitions, so reciprocal + one multiply
                    rb = stats.tile([P, T], FP32, tag="rb", bufs=2,
                                    name="rb")
                    nc.vector.reciprocal_approx_fast(rb, cs_ps)
                    nc.vector.tensor_mul(ot_sb[:, h, b], o_ps, rb)
                    del head_state[(b, h)]

            def emit_att(b, h):
                emit_item_S(b, h)
                while len(pend) > 11:
                    drain_one()

            def emit_oproj(tt, cchunk):
                # one out-projection chunk: out[tt-block, chunk] as its own
                # k-chain, sharing the qps psum rotation with the (finished)
                # Q chains so it can interleave with the attention tail
                ps = psM.tile([P, T], FP32, tag="qps", bufs=2, name="opps")
                for k in range(KT_HID):
                    nc.tensor.matmul(
                        ps,
                        ot_sb[:, k, tt // NTQ,
                              (tt % NTQ) * P:(tt % NTQ + 1) * P],
                        wo_sb[:, k, cchunk * T:(cchunk + 1) * T],
                        start=(k == 0), stop=(k == KT_HID - 1))
                o_tile = stats.tile([P, T], BF16, tag="oout", bufs=3,
                                    name="o_tile")
                if tt == NTOK_T - 1 and cchunk == 1:
                    # very last chunk: ONE full-width DVE copy + one sync
                    # store. Psum-read copies are latency-bound (width is
                    # nearly free), and the DVE wakes ~0.6us earlier than
                    # ACT after the final matmul stops -- a single earliest-
                    # starting copy beats any parallel split
                    base = cchunk * T
                    nc.vector.tensor_copy(o_tile, ps)
                    nc.sync.dma_start(
                        out=out[tt * P:(tt + 1) * P, base:base + T],
                        in_=o_tile)
                    return
                # copies always on ACT: a PE-dependent copy on the DVE would
                # head-of-line delay the recips/norms that gate later chains;
                # stores always on the otherwise-idle sync queue
                nc.scalar.copy(o_tile, ps)
                nc.sync.dma_start(
                    out=out[tt * P:(tt + 1) * P,
                            cchunk * T:(cchunk + 1) * T],
                    in_=o_tile)

            # attention lags the Q chains by 3 heads and the DVE part of
            # each rope lags its chain by 1 slot: every engine-queue entry
            # has its dependencies resolved before it reaches the queue
            # head, so the strict-FIFO DVE queue never head-of-line blocks
            # the causal masks that gate the PE's colsum/PV matmuls
            LAG = 3
            for h in range(NH):
                if h >= LAG:
                    emit_att(0, h - LAG)
                emit_qchain(h, 0)
                if h >= LAG:
                    emit_att(1, h - LAG)
                if rope_pending and h >= 1:
                    emit_rope_dve()
                emit_qchain(h, 1)
            while rope_pending:
                emit_rope_dve()
            # tail: batch-0 attention first, then interleave out-projection
            # chunks (batch-0 token tiles first) with the remaining
            # ACT/DVE-bound attention so the PE stays dense
            for h in range(NH - LAG, NH):
                emit_att(0, h)
            emit_att(1, NH - LAG)
            while len(pend) > 4:
                drain_one()              # flush: norms (0, *) all emitted
            emit_oproj(0, 0)
            emit_oproj(0, 1)
            emit_att(1, NH - 2)
            emit_oproj(1, 0)
            emit_oproj(1, 1)
            emit_att(1, NH - 1)
            emit_oproj(2, 0)
            emit_oproj(2, 1)
            while len(pend) > 4:
                drain_one()              # flush: norm (1, NH-2) emitted
            emit_oproj(3, 0)
            while pend:
                drain_one()              # norm (1, NH-1)
            emit_oproj(3, 1)
            for tt in range(4, NTOK_T):
                emit_oproj(tt, 0)
                emit_oproj(tt, 1)


def _prep_weights(Wq, Wk, Wv, Wo):
    """Host-side: bf16, rope-pair permutation of the q/k head dims, and
    rearrangement to the sbuf fill layout [p, k, n] (contiguous per
    partition) so load DMA descriptors are large."""
    wq = np.asarray(Wq, dtype=np.float32).astype(BF)
    wk = np.asarray(Wk, dtype=np.float32).astype(BF)
    wv = np.asarray(Wv, dtype=np.float32).astype(BF)
    wo = np.asarray(Wo, dtype=np.float32).astype(BF)
    # permute the per-head d axis of Wq/Wk
    wq = wq.reshape(HID, NH, D)[:, :, _DPERM].reshape(HID, NH * D)
    wk = wk.reshape(HID, NKV, D)[:, :, _DPERM].reshape(HID, NKV * D)
    wq_pk = np.ascontiguousarray(
        wq.reshape(KT_HID, P, NH * D).transpose(1, 0, 2))
    wk_g = np.ascontiguousarray(
        wk.reshape(KT_HID, P, NKV, D).transpose(2, 1, 0, 3))
    wv_pk = np.ascontiguousarray(
        wv.reshape(KT_HID, P, NKV * D).transpose(1, 0, 2))
    wo_pk = np.ascontiguousarray(
        wo.reshape(KT_HID, P, HID).transpose(1, 0, 2))
    return {"wq_pk": wq_pk, "wk_g": wk_g, "wv_pk": wv_pk, "wo_pk": wo_pk}


_COMPILED = None


def _get_compiled():
    global _COMPILED
    if _COMPILED is None:
        nc = bacc.Bacc("TRN2", target_bir_lowering=False, debug=False)
        _build(nc)
        nc.compile()
        _COMPILED = nc
    return _COMPILED


def kernel(hidden_states, Wq, Wk, Wv, Wo, _trace=False, _trace_kwargs=None):
    hs = np.asarray(hidden_states, dtype=np.float32).astype(BF)
    weights = _prep_weights(Wq, Wk, Wv, Wo)
    consts = _host_consts()
    nc = _get_compiled()
    in_maps = []
    for c in range(NCORES):
        # ship X pre-transposed AND pre-tiled ([p, k, tok]) so the kernel's
        # loads are plane-sliced contiguous DMAs
        shard = hs[BL * c: BL * (c + 1)].reshape(TOK, HID).T  # [HID, TOK]
        shard_pk = np.ascontiguousarray(
            shard.reshape(KT_HID, P, TOK).transpose(1, 0, 2))
        in_maps.append({"hidden_pk": shard_pk, **weights, **consts})
    res = run_bass_kernel_spmd(
        nc, in_maps, list(range(NCORES)), trace=_trace,
        **(_trace_kwargs or {}))
    outs = [np.asarray(r["out"]).astype(np.float32).reshape(BL, T, HID)
            for r in res.results]
    full = np.concatenate(outs, axis=0)
    if _trace:
        return full, res
    return full
